# revision 1
# baseline (speedup 1.0000x reference)
"""ChebConv (K=6) message-passing kernel for 8 Trainium2 NeuronCores.

Math: the reference's GraphNetwork pass multiplies each node's features by a
per-node scalar s = (deg - in_w) / max(deg) (deg = segment_sum(edges, senders),
in_w = segment_sum(edges, receivers)), and the Chebyshev recurrence
Tx_k = 2*Tx_{k-1} - Tx_{k-2} stays rank-1 per node: Tx_k = (1 + k*(s-1)) * x.
Hence
    out = X @ WA + s * (X @ WB) + b_tot
with WA = sum_k (1-k) Wk[k], WB = sum_k k Wk[k], b_tot = sum_k bk[k] + bias.

Sharding: nodes are block-sharded over 8 cores (12500 each, padded to 12544).
Edges are routed on the host (index permutation + zero fill only, no float
arithmetic) to the core owning their sender (for deg) / receiver (for in_w),
laid out as a per-node padded slot matrix so each segment-sum becomes a dense
row reduction on device.

Two launches (an in-kernel AllReduce costs ~60us of comm-subsystem init, far
more than a second NEFF):
  A: edge kernel — per-core deg/in_w row-reductions + per-core max(deg).
  host: m = max of the 8 device-computed partial maxima (selection only).
  B: main kernel — s = (deg-in_w)*recip(m), X@[WA|WB] matmuls (fp32r),
     s-scaled combine + bias, all overlapped with DMA.
"""

import sys

sys.path.insert(0, "/opt/trn_rl_repo")

import numpy as np

import concourse.bacc as bacc
import concourse.bass as bass
import concourse.mybir as mybir
import concourse.tile as tile
from concourse import bass_isa
from concourse.bass_utils import run_bass_kernel_spmd

N_NODES = 100000
F = 128
KCH = 6
NCORES = 8
NPC = N_NODES // NCORES       # 12500 nodes per core
T = (NPC + 127) // 128        # 98 node tiles per core
NPAD = T * 128                # 12544 (rows 12500.. are zero padding)
DSLOT_MIN = 64                # per-node edge-slot padding (>= max degree)

f32 = mybir.dt.float32
f32r = mybir.dt.float32r
bf16 = mybir.dt.bfloat16
fp16 = mybir.dt.float16

# test.py knobs (harness never touches these)
TRACE = False
LAST = {}
MM_MODE = "tfp16"  # "f32" | "f32r" | "tfp16" | "tf32r"

_prog_cache = {}


def _build_edge_program(ds, dr):
    """Launch A: deg = rowsum(pse), inw = rowsum(pre), pmax = max(deg)."""
    nc = bacc.Bacc("TRN2", target_bir_lowering=False, debug=False,
                   num_devices=NCORES)
    A = mybir.AluOpType
    X = mybir.AxisListType.X

    pse_d = nc.dram_tensor("pse", [128, T * ds], fp16, kind="ExternalInput")
    pre_d = nc.dram_tensor("pre", [128, T * dr], fp16, kind="ExternalInput")
    degw_d = nc.dram_tensor("degw", [128, 2 * T], f32, kind="ExternalOutput")
    pmax_d = nc.dram_tensor("pmax", [1, 1], f32, kind="ExternalOutput")

    CH = [(0, 25), (25, 25), (50, 24), (74, 24)]
    with tile.TileContext(nc) as tc:
        with (
            tc.tile_pool(name="edge", bufs=1) as edgep,
            tc.tile_pool(name="small", bufs=1) as smallp,
        ):
            degw = smallp.tile([128, 2 * T], f32)
            pse_sb = edgep.tile([128, T, ds], fp16)
            pre_sb = edgep.tile([128, T, dr], fp16)
            for t0, n in CH:
                nc.sync.dma_start(
                    pse_sb[:, t0 : t0 + n, :],
                    pse_d[:, t0 * ds : (t0 + n) * ds].rearrange("p (t d) -> p t d", d=ds))
                nc.vector.tensor_reduce(degw[:, t0 : t0 + n], pse_sb[:, t0 : t0 + n, :],
                                        axis=X, op=A.add)
            for t0, n in CH:
                nc.sync.dma_start(
                    pre_sb[:, t0 : t0 + n, :],
                    pre_d[:, t0 * dr : (t0 + n) * dr].rearrange("p (t d) -> p t d", d=dr))
                nc.vector.tensor_reduce(degw[:, T + t0 : T + t0 + n], pre_sb[:, t0 : t0 + n, :],
                                        axis=X, op=A.add)

            dmax = smallp.tile([128, 1], f32)
            nc.vector.tensor_reduce(dmax[:, :], degw[:, :T], axis=X, op=A.max)
            pmax = smallp.tile([128, 1], f32)
            nc.gpsimd.partition_all_reduce(pmax[:, :], dmax[:, :], channels=128,
                                           reduce_op=bass_isa.ReduceOp.max)
            nc.sync.dma_start(degw_d[:, :], degw[:, :])
            nc.sync.dma_start(pmax_d[:, :], pmax[0:1, 0:1])

    nc.compile()
    return nc


def _build_main_program(mm_mode):
    """Launch B: out = X@WA + s*(X@WB) + b_tot, s = (deg-inw)*recip(m)."""
    nc = bacc.Bacc("TRN2", target_bir_lowering=False, debug=False,
                   num_devices=NCORES)
    A = mybir.AluOpType
    X = mybir.AxisListType.X
    mmdt = f32r if mm_mode == "f32r" else f32

    xt_d = nc.dram_tensor("xt", [F, NPAD], mmdt, kind="ExternalInput")
    wk_d = nc.dram_tensor("wk", [KCH, F, F], f32, kind="ExternalInput")
    bkb_d = nc.dram_tensor("bkb", [1, (KCH + 1) * F], f32, kind="ExternalInput")
    degw_d = nc.dram_tensor("degw", [128, 2 * T], f32, kind="ExternalInput")
    mmax_d = nc.dram_tensor("mmax", [1, 1], f32, kind="ExternalInput")
    out_d = nc.dram_tensor("out", [NPAD, F], f32, kind="ExternalOutput")

    XCH = 7                    # xt / out DMA chunks
    TCH = T // XCH             # 14 node tiles per chunk
    G = 7                      # node tiles per PSUM group (7KB -> 4 banks)

    with tile.TileContext(nc) as tc:
        with (
            tc.tile_pool(name="const", bufs=1) as constp,
            tc.tile_pool(name="xt", bufs=1) as xtp,
            tc.tile_pool(name="outp", bufs=1) as outp,
            tc.tile_pool(name="small", bufs=1) as smallp,
            tc.tile_pool(name="tmp", bufs=4) as tmpp,
            tc.tile_pool(name="ps", bufs=2, space="PSUM") as psp,
        ):
            # ---- constants (tiny DMAs first so wab is ready early) --------
            wk_sb = constp.tile([128, KCH, F], f32)
            nc.sync.dma_start(wk_sb[:, :, :], wk_d.ap().rearrange("k p f -> p k f"))
            bkb_sb = constp.tile([1, (KCH + 1) * F], f32)
            nc.sync.dma_start(bkb_sb[:, :], bkb_d[:, :])
            degw_sb = smallp.tile([128, 2 * T], f32)
            nc.sync.dma_start(degw_sb[:, :], degw_d[:, :])

            # WA | WB:  WA = W0 - W2 - 2W3 - 3W4 - 4W5,
            #           WB = W1 + 2W2 + 3W3 + 4W4 + 5W5
            wab = constp.tile([128, 2 * F], mmdt)
            wa, wb = wab[:, 0:F], wab[:, F : 2 * F]
            nc.vector.scalar_tensor_tensor(wa, wk_sb[:, 2, :], -1.0, wk_sb[:, 0, :], op0=A.mult, op1=A.add)
            nc.vector.scalar_tensor_tensor(wa, wk_sb[:, 3, :], -2.0, wa, op0=A.mult, op1=A.add)
            nc.vector.scalar_tensor_tensor(wa, wk_sb[:, 4, :], -3.0, wa, op0=A.mult, op1=A.add)
            nc.vector.scalar_tensor_tensor(wa, wk_sb[:, 5, :], -4.0, wa, op0=A.mult, op1=A.add)
            nc.vector.scalar_tensor_tensor(wb, wk_sb[:, 2, :], 2.0, wk_sb[:, 1, :], op0=A.mult, op1=A.add)
            nc.vector.scalar_tensor_tensor(wb, wk_sb[:, 3, :], 3.0, wb, op0=A.mult, op1=A.add)
            nc.vector.scalar_tensor_tensor(wb, wk_sb[:, 4, :], 4.0, wb, op0=A.mult, op1=A.add)
            nc.vector.scalar_tensor_tensor(wb, wk_sb[:, 5, :], 5.0, wb, op0=A.mult, op1=A.add)

            # b_tot broadcast to all partitions via a K=1 outer-product matmul
            btot = smallp.tile([1, F], f32)
            nc.vector.tensor_reduce(
                btot[:, :], bkb_sb.rearrange("p (s f) -> p f s", s=KCH + 1),
                axis=X, op=A.add)
            btot_b = smallp.tile([1, F], bf16)
            nc.vector.tensor_copy(btot_b[:, :], btot[:, :])
            ones_row = smallp.tile([1, F], bf16)
            nc.vector.memset(ones_row[:, :], 1.0)
            btile_ps = psp.tile([128, F], f32, tag="ps")
            nc.tensor.matmul(btile_ps[:, :], ones_row[:, :], btot_b[:, :],
                             start=True, stop=True)
            btile = smallp.tile([128, F], f32)
            nc.vector.tensor_copy(btile[:, :], btile_ps[:, :])

            # ---- s = (deg - inw) * recip(m) -------------------------------
            m_bc = smallp.tile([128, 1], f32)
            map_ = mmax_d[0:1, 0:1]
            nc.sync.dma_start(m_bc[:, :], bass.AP(map_.tensor, map_.offset, [[0, 128], [1, 1]]))
            minv = smallp.tile([128, 1], f32)
            nc.vector.reciprocal(minv[:, :], m_bc[:, :])
            s_sb = smallp.tile([128, T], f32)
            nc.vector.tensor_sub(s_sb[:, :], degw_sb[:, :T], degw_sb[:, T :])
            nc.vector.tensor_scalar_mul(s_sb[:, :], s_sb[:, :], minv[:, 0:1])

            # ---- node features -------------------------------------------
            xt_sb = []
            for c in range(XCH):
                xt_c = xtp.tile([128, TCH * 128], mmdt, name=f"xt{c}")
                nc.sync.dma_start(xt_c[:, :], xt_d[:, c * TCH * 128 : (c + 1) * TCH * 128])
                xt_sb.append(xt_c)

            # ---- matmuls + combine ---------------------------------------
            def bc(ap, reps, width):
                """[128, G] tile slice -> [128, G, width] 0-stride broadcast AP."""
                return bass.AP(ap.tensor, ap.offset, [ap.ap[0], [1, reps], [0, width]])

            for c in range(XCH):
                out_c = outp.tile([128, TCH, F], f32, name=f"out{c}")
                for g in range(TCH // G):
                    t0 = c * TCH + g * G
                    ps = psp.tile([128, G, 2 * F], f32, tag="ps")
                    for u in range(G):
                        j = g * G + u
                        nc.tensor.matmul(ps[:, u, :],
                                         xt_sb[c][:, j * 128 : (j + 1) * 128],
                                         wab[:, :], start=True, stop=True)
                    tmp = tmpp.tile([128, G, F], f32, tag="tmp")
                    g_abs = c * (TCH // G) + g
                    if g_abs % 7 < 5:
                        # scalar engine: per-tile copy with per-partition scale
                        for u in range(G):
                            nc.scalar.activation(tmp[:, u, :], ps[:, u, F : 2 * F],
                                                 mybir.ActivationFunctionType.Copy,
                                                 scale=s_sb[:, t0 + u : t0 + u + 1])
                    else:
                        nc.vector.tensor_tensor(tmp[:, :, :], ps[:, :, F : 2 * F],
                                                bc(s_sb[:, t0 : t0 + G], G, F),
                                                op=A.mult)
                    nc.vector.tensor_tensor(out_c[:, g * G : (g + 1) * G, :],
                                            tmp[:, :, :], ps[:, :, 0:F], op=A.add)
                bt = btile[:, :]
                btile_bc = bass.AP(bt.tensor, bt.offset, [bt.ap[0], [0, TCH], [1, F]])
                nc.vector.tensor_tensor(out_c[:, :, :], out_c[:, :, :], btile_bc, op=A.add)
                nc.sync.dma_start(
                    out_d[c * TCH * 128 : (c + 1) * TCH * 128, :].rearrange("(j p) f -> p j f", p=128),
                    out_c[:, :, :])

    nc.compile()
    return nc


def _build_main_program_t(mm_mode):
    """Launch B (transposed): outT = WA.T@X.T + WB.T@(s*X).T + b_tot, where
    X.T arrives feature-major ([fi, n]) so s varies along the free axis.  The
    s-scaled term is a second accumulating matmul with sx = x*srep; srep is a
    PE outer-product broadcast of s.  Bias rides the ACT evacuation as a
    per-partition bias (fo is the partition axis here).  Host transposes the
    [F, NPAD] result back."""
    nc = bacc.Bacc("TRN2", target_bir_lowering=False, debug=False,
                   num_devices=NCORES)
    A = mybir.AluOpType
    X = mybir.AxisListType.X
    mmdt = fp16 if mm_mode == "tfp16" else f32r

    xt_d = nc.dram_tensor("xt", [F, NPAD], mmdt, kind="ExternalInput")
    wk_d = nc.dram_tensor("wk", [KCH, F, F], f32, kind="ExternalInput")
    bkb_d = nc.dram_tensor("bkb", [1, (KCH + 1) * F], f32, kind="ExternalInput")
    degw_d = nc.dram_tensor("degw", [128, 2 * T], f32, kind="ExternalInput")
    mmax_d = nc.dram_tensor("mmax", [1, 1], f32, kind="ExternalInput")
    odt = fp16 if mm_mode == "tfp16" else f32
    out_d = nc.dram_tensor("out", [F, NPAD], odt, kind="ExternalOutput")

    XCH = 7                    # xt / out DMA chunks (1792 cols each)
    CW = NPAD // XCH           # 1792
    GW = 448                   # matmul group width (PSUM bank = 512 f32 max)
    GPC = CW // GW             # 4 groups per chunk

    from concourse import masks

    with tile.TileContext(nc) as tc:
        with (
            tc.tile_pool(name="const", bufs=1) as constp,
            tc.tile_pool(name="xt", bufs=1) as xtp,
            tc.tile_pool(name="outp", bufs=1) as outp,
            tc.tile_pool(name="small", bufs=1) as smallp,
            tc.tile_pool(name="sx", bufs=5) as sxp,
            tc.tile_pool(name="srepp", bufs=1) as srepp,
            tc.tile_pool(name="psf", bufs=7, space="PSUM") as psf,
            tc.tile_pool(name="pst", bufs=1, space="PSUM") as pst,
            tc.tile_pool(name="dram", bufs=1, space="DRAM") as dramp,
        ):
            # ---- tiny input DMAs (m + degw first: they feed the s chain) --
            with tc.high_priority():
                m_bc = smallp.tile([128, 1], f32)
                map_ = mmax_d[0:1, 0:1]
                nc.sync.dma_start(m_bc[:, :], bass.AP(map_.tensor, map_.offset, [[0, 128], [1, 1]]))
                degw_sb = smallp.tile([128, 2 * T], f32)
                nc.sync.dma_start(degw_sb[:, :], degw_d[:, :])
            wk_sb = constp.tile([128, KCH, F], f32)
            nc.sync.dma_start(wk_sb[:, :, :], wk_d.ap().rearrange("k p f -> p k f"))
            bkb_sb = constp.tile([1, (KCH + 1) * F], f32)
            nc.sync.dma_start(bkb_sb[:, :], bkb_d[:, :])

            # ---- weights: WA | WB in f32, then cast to matmul dtype --------
            wab = constp.tile([128, 2 * F], f32)
            wa, wb = wab[:, 0:F], wab[:, F : 2 * F]
            nc.vector.scalar_tensor_tensor(wa, wk_sb[:, 2, :], -1.0, wk_sb[:, 0, :], op0=A.mult, op1=A.add)
            nc.vector.scalar_tensor_tensor(wa, wk_sb[:, 3, :], -2.0, wa, op0=A.mult, op1=A.add)
            nc.vector.scalar_tensor_tensor(wa, wk_sb[:, 4, :], -3.0, wa, op0=A.mult, op1=A.add)
            nc.vector.scalar_tensor_tensor(wa, wk_sb[:, 5, :], -4.0, wa, op0=A.mult, op1=A.add)
            nc.vector.scalar_tensor_tensor(wb, wk_sb[:, 2, :], 2.0, wk_sb[:, 1, :], op0=A.mult, op1=A.add)
            nc.vector.scalar_tensor_tensor(wb, wk_sb[:, 3, :], 3.0, wb, op0=A.mult, op1=A.add)
            nc.vector.scalar_tensor_tensor(wb, wk_sb[:, 4, :], 4.0, wb, op0=A.mult, op1=A.add)
            nc.vector.scalar_tensor_tensor(wb, wk_sb[:, 5, :], 5.0, wb, op0=A.mult, op1=A.add)
            wa16 = constp.tile([128, F], mmdt)
            wb16 = constp.tile([128, F], mmdt)
            nc.vector.tensor_copy(wa16[:, :], wa)
            nc.vector.tensor_copy(wb16[:, :], wb)

            # ---- s = (deg - inw) * recip(m), transposed to node order ------
            with tc.high_priority():
                minv = smallp.tile([128, 1], f32)
                nc.vector.reciprocal(minv[:, :], m_bc[:, :])
                s_sb = smallp.tile([128, T], f32)
                nc.vector.tensor_sub(s_sb[:, :], degw_sb[:, :T], degw_sb[:, T:])
                nc.vector.tensor_scalar_mul(s_sb[:, :], s_sb[:, :], minv[:, 0:1])
                s16 = smallp.tile([128, 128], fp16)
                nc.vector.memset(s16[:, :], 0.0)
                nc.vector.tensor_copy(s16[:, 0:T], s_sb[:, :])
                ident16 = smallp.tile([128, 128], fp16)
                masks.make_identity(nc, ident16[:, :])
            with tc.high_priority():
                ps_t = pst.tile([128, 128], fp16, tag="pst")
                nc.tensor.transpose(ps_t[:, :], s16[:, :], ident16[:, :])
                s_tr = smallp.tile([128, 128], fp16)
                nc.vector.tensor_copy(s_tr[:, :], ps_t[:, :])
                strow_d = dramp.tile([T, 128], fp16)
                nc.sync.dma_start(strow_d[:, :], s_tr[0:T, :])


            # ---- bias as a column (per-partition in transposed space) ------
            with tc.high_priority():
                # table pre-warm so the first real ACT op isn't stuck behind
                # the one-time activation-table load
                act_warm = smallp.tile([1, 1], f32)
                nc.scalar.activation(act_warm[:, :], m_bc[0:1, 0:1],
                                     mybir.ActivationFunctionType.Identity,
                                     bias=0.0, scale=1.0)
                btot = smallp.tile([1, F], f32)
                nc.vector.tensor_reduce(
                    btot[:, :], bkb_sb.rearrange("p (s f) -> p f s", s=KCH + 1),
                    axis=X, op=A.add)
                one1 = smallp.tile([1, 1], f32)
                nc.vector.memset(one1[:, :], 1.0)
                ps_bc = pst.tile([128, 1], f32, tag="pst")
                nc.tensor.matmul(ps_bc[:, :], btot[:, :], one1[:, :],
                                 start=True, stop=True)
                btot_col = smallp.tile([128, 1], f32)
                nc.vector.tensor_copy(btot_col[:, :], ps_bc[:, :])

            # ---- node features + s broadcasts (all prefetched; chunk 0 is
            # split into halves so the matmul stream starts sooner) ----------
            sflat = strow_d[:, :]
            xt_sb, srep_sb = [], []
            for c in range(XCH):
                xt_c = xtp.tile([128, CW], mmdt, name=f"xt{c}")
                if c == 0:
                    with tc.high_priority():
                        H2 = CW // 2
                        nc.sync.dma_start(xt_c[:, :H2], xt_d[:, :H2])
                        nc.sync.dma_start(xt_c[:, H2:], xt_d[:, H2:CW])
                else:
                    nc.sync.dma_start(xt_c[:, :], xt_d[:, c * CW : (c + 1) * CW])
                xt_sb.append(xt_c)
                # broadcast DMA: every partition re-reads this chunk's
                # node-ordered s row from DRAM
                srep_c = srepp.tile([128, CW], fp16, name=f"srep{c}")
                srcap = bass.AP(sflat.tensor, sflat.offset + c * CW, [[0, 128], [1, CW]])
                nc.sync.dma_start(srep_c[:, :], srcap)
                srep_sb.append(srep_c)

            # ---- main loop -------------------------------------------------
            for c in range(XCH):
                out_c = outp.tile([128, CW], odt, name=f"out{c}")
                srep_c = srep_sb[c]
                for gp in range(GPC // 4):
                    # quad groups: stationary sequence wa x4, wb x4
                    n0s = [(4 * gp + i) * GW for i in range(4)]
                    sxs, psFs = [], []
                    for n0 in n0s:
                        sx = sxp.tile([128, GW], mmdt, tag="sx")
                        nc.vector.tensor_tensor(sx[:, :], xt_sb[c][:, n0 : n0 + GW],
                                                srep_c[:, n0 : n0 + GW], op=A.mult)
                        sxs.append(sx)
                    for n0 in n0s:
                        psF = psf.tile([128, GW], f32, tag="psf")
                        nc.tensor.matmul(psF[:, :], wa16[:, :],
                                         xt_sb[c][:, n0 : n0 + GW], start=True, stop=False)
                        psFs.append(psF)
                    for n0, sx, psF in zip(n0s, sxs, psFs):
                        nc.tensor.matmul(psF[:, :], wb16[:, :], sx[:, :],
                                         start=False, stop=True)
                    for n0, psF in zip(n0s, psFs):
                        nc.scalar.activation(out_c[:, n0 : n0 + GW], psF[:, :],
                                             mybir.ActivationFunctionType.Identity,
                                             bias=btot_col[:, 0:1], scale=1.0)
                nc.sync.dma_start(out_d[:, c * CW : (c + 1) * CW], out_c[:, :])

    nc.compile()
    return nc


def _ceil8(x):
    return max(DSLOT_MIN, (int(x) + 7) // 8 * 8)


def _route_edges(vals, idx, dslot):
    """Host-side edge routing: permutation + zero-fill only (layout for the
    device segment-sum; no float arithmetic happens here)."""
    order = np.argsort(idx, kind="stable")
    si = idx[order]
    sv = vals[order]
    cnt = np.bincount(idx, minlength=N_NODES)
    first = np.concatenate(([0], np.cumsum(cnt)[:-1]))
    slot = np.arange(idx.shape[0], dtype=np.int64) - first[si]
    core = si // NPC
    ln = si - core * NPC
    rows = ln % 128
    cols = (ln // 128) * dslot + slot
    packed = np.zeros((NCORES, 128, T * dslot), np.float16)
    packed[core, rows, cols] = sv
    return packed


def kernel(nodes, edges, senders, receivers, Wk, bk, bias):
    nodes = np.ascontiguousarray(np.asarray(nodes, np.float32))
    edges = np.ascontiguousarray(np.asarray(edges, np.float32))
    senders = np.asarray(senders)
    receivers = np.asarray(receivers)
    Wk = np.ascontiguousarray(np.asarray(Wk, np.float32))
    bk = np.asarray(bk, np.float32)
    bias = np.asarray(bias, np.float32)
    assert nodes.shape == (N_NODES, F) and Wk.shape == (KCH, F, F)

    ds = _ceil8(np.bincount(senders, minlength=N_NODES).max())
    dr = _ceil8(np.bincount(receivers, minlength=N_NODES).max())

    if ("edge", ds, dr) not in _prog_cache:
        _prog_cache[("edge", ds, dr)] = _build_edge_program(ds, dr)
    if ("main", MM_MODE) not in _prog_cache:
        if MM_MODE.startswith("t"):
            _prog_cache[("main", MM_MODE)] = _build_main_program_t(MM_MODE)
        else:
            _prog_cache[("main", MM_MODE)] = _build_main_program(MM_MODE)
    ncA = _prog_cache[("edge", ds, dr)]
    ncB = _prog_cache[("main", MM_MODE)]
    transposed = MM_MODE.startswith("t")

    pse = _route_edges(edges, senders, ds)
    pre = _route_edges(edges, receivers, dr)
    bkb = np.ascontiguousarray(
        np.concatenate([bk.reshape(1, -1), bias.reshape(1, -1)], axis=1), np.float32)

    cores = list(range(NCORES))
    in_a = [{"pse": np.ascontiguousarray(pse[c]),
             "pre": np.ascontiguousarray(pre[c])} for c in cores]
    res_a = run_bass_kernel_spmd(ncA, in_a, cores, trace=TRACE)

    # combine the 8 device-computed partial maxima (selection, no arithmetic)
    m = max(float(res_a.results[c]["pmax"][0, 0]) for c in cores)
    mmax = np.array([[m]], np.float32)

    xdt = np.float16 if MM_MODE == "tfp16" else np.float32
    in_b = []
    for c in cores:
        xt = np.zeros((F, NPAD), xdt)
        xt[:, :NPC] = nodes[c * NPC : (c + 1) * NPC].T
        in_b.append({
            "xt": xt,
            "wk": Wk,
            "bkb": bkb,
            "degw": res_a.results[c]["degw"],
            "mmax": mmax,
        })
    res_b = run_bass_kernel_spmd(ncB, in_b, cores, trace=TRACE)

    ta = res_a.exec_time_ns
    tb = res_b.exec_time_ns
    LAST["exec_a_ns"] = ta
    LAST["exec_b_ns"] = tb
    LAST["exec_time_ns"] = (ta + tb) if (ta is not None and tb is not None) else None

    out = np.empty((N_NODES, F), np.float32)
    for c in cores:
        o = res_b.results[c]["out"]
        if transposed:
            out[c * NPC : (c + 1) * NPC] = o.astype(np.float32).T[:NPC]
        else:
            out[c * NPC : (c + 1) * NPC] = o[:NPC]
    return out



# revision 11
# speedup vs baseline: 1.0976x; 1.0976x over previous
"""ChebConv (K=6) message-passing kernel for 8 Trainium2 NeuronCores.

Math: the reference's GraphNetwork pass multiplies each node's features by a
per-node scalar s = (deg - in_w) / max(deg) (deg = segment_sum(edges, senders),
in_w = segment_sum(edges, receivers)), and the Chebyshev recurrence
Tx_k = 2*Tx_{k-1} - Tx_{k-2} stays rank-1 per node: Tx_k = (1 + k*(s-1)) * x.
Hence
    out = X @ WA + s * (X @ WB) + b_tot
with WA = sum_k (1-k) Wk[k], WB = sum_k k Wk[k], b_tot = sum_k bk[k] + bias.

Sharding: nodes are block-sharded over 8 cores (12500 each, padded to 12544).
Edges are routed on the host (index permutation + zero fill only, no float
arithmetic) to the core owning their sender (for deg) / receiver (for in_w).

Two launches (an in-kernel AllReduce costs ~60us of comm-subsystem init, far
more than a second NEFF):
  A: edge kernel -- segment sums as PE mask-matmuls.  Edges are packed on the
     host into fp8 columns of 128 slots; nodes are degree-classed (slot count
     rounded to a multiple of 8) so a column holds k = 128//class nodes at
     fixed offsets.  One [128,32] 0/1 mask per class turns a 512-column chunk
     into per-node sums via a single matmul per PSUM quadrant
     (tile_position col-tiling).  DVE max-reduce + gpsimd partition reduce
     give the per-core max(deg).
  host: m = max of the 8 device partial maxima (selection only); deg/in_w
     values are host-permuted (selection) from packed order to node order.
  B: main kernel -- s = (deg-inw)*recip(m); transposed layout outT[fo, n]:
     psF = WA^T @ X + WB^T @ (s*X) accumulated in PSUM, evacuated by ACT with
     per-partition bias; s broadcast along partitions via an fp8 DRAM
     broadcast read (srep).
"""

import sys

sys.path.insert(0, "/opt/trn_rl_repo")

import numpy as np
import ml_dtypes

import concourse.bacc as bacc
import concourse.bass as bass
import concourse.mybir as mybir
import concourse.tile as tile
from concourse import bass_isa, masks
from concourse.bass_utils import run_bass_kernel_spmd

N_NODES = 100000
F = 128
KCH = 6
NCORES = 8
NPC = N_NODES // NCORES       # 12500 nodes per core
T = (NPC + 127) // 128        # 98 node tiles per core
NPAD = T * 128                # 12544 (cols 12500.. are zero padding)

f32 = mybir.dt.float32
bf16 = mybir.dt.bfloat16
fp16 = mybir.dt.float16
fp8 = mybir.dt.float8e4

np_fp8 = ml_dtypes.float8_e4m3fn

# test.py knobs (harness never touches these)
TRACE = False
LAST = {}

PW = 512                      # PSUM chunk width (one bank of f32)

_prog_cache = {}


# --------------------------------------------------------------------------
# host-side edge packing for the PE segment-sum (permutation + zero-fill only)
# --------------------------------------------------------------------------

class _CommonPlan:
    """Shared (all-cores) packing layout for one buffer (sender- or
    receiver-keyed edges).

    Column layout: a column has 128 slots; it holds k = 128//cls nodes of the
    same degree-class cls (cls = ceil8(cnt)), node i at slot rows
    [i*cls, i*cls + cnt).  Per-class column counts are the max over cores so
    one program serves all cores (unused columns stay zero).  Columns are
    grouped into pieces of <= PW columns (one matmul each), pieces into
    chunks of <= 4 (PSUM quadrants).
    """

    def __init__(self, cls_per_core):
        classes = set()
        for cls in cls_per_core:
            classes.update(int(c) for c in np.unique(cls))
        self.classes = sorted(classes)
        self.colbase_cl = []       # virtual (class-contiguous) column base
        vcol0 = 0
        pieces = []  # [cls_idx, vcol0, ncols]
        for ci, cl in enumerate(self.classes):
            k = 128 // cl
            n = max(int((cls == cl).sum()) for cls in cls_per_core)
            ncol = (n + k - 1) // k
            self.colbase_cl.append(vcol0)
            p0 = 0
            while p0 < ncol:
                w = min(PW, ncol - p0)
                pieces.append([ci, vcol0 + p0, w])
                p0 += w
            vcol0 += ncol
        self.nvcols = vcol0
        # chunks: group pieces (desc by width) into groups of 4; every piece
        # occupies a full chunk-width data stripe (zero-padded) so the matmul
        # covers the whole PSUM quadrant (stale-PSUM-free max/evac).
        pieces.sort(key=lambda p: -p[2])
        self.chunks = []   # list of (W, [(cls_idx, padded_base, vcol0, w), ...])
        ncols = 0
        for i in range(0, len(pieces), 4):
            grp = pieces[i : i + 4]
            W = grp[0][2]
            g2 = []
            for ci, v0, w in grp:
                g2.append((ci, ncols, v0, w))
                ncols += W
            self.chunks.append((W, g2))
        self.ncols = ncols
        self.evac_cols = sum(W for W, _ in self.chunks)
        # piece lookup tables for vcol -> padded data col
        self._pv0 = np.array([p[2] for _, g in self.chunks for p in g], np.int64)
        self._ppb = np.array([p[1] for _, g in self.chunks for p in g], np.int64)

    def _v2d(self, vcol):
        """Map virtual cols to padded data cols (vectorized)."""
        # piece index: each piece covers [v0, v0+w) and pieces are disjoint
        order = np.argsort(self._pv0)
        v0s = self._pv0[order]
        pbs = self._ppb[order]
        pi = np.searchsorted(v0s, vcol, side="right") - 1
        return pbs[pi] + (vcol - v0s[pi])

    def fill(self, data, colbase, ln, vals, NPCL):
        """Scatter edge values into data[:, colbase:...]; returns per-node
        (evac_partition, evac_col) for reading packed sums back."""
        cnt = np.bincount(ln, minlength=NPCL)
        assert cnt.max() <= 128, f"node degree {cnt.max()} > 128 unsupported"
        cls = np.maximum(((np.maximum(cnt, 1) + 7) // 8) * 8, 8)
        node_vcol = np.zeros(NPCL, np.int64)
        node_row = np.zeros(NPCL, np.int64)
        for ci, cl in enumerate(self.classes):
            nodes = np.nonzero(cls == cl)[0]
            if not len(nodes):
                continue
            k = 128 // cl
            j = np.arange(len(nodes))
            node_vcol[nodes] = self.colbase_cl[ci] + j // k
            node_row[nodes] = (j % k) * cl
        node_dcol = self._v2d(node_vcol)
        order = np.argsort(ln, kind="stable")
        se = ln[order]
        sv = vals[order]
        first = np.concatenate(([0], np.cumsum(cnt)[:-1]))
        slot = np.arange(len(se), dtype=np.int64) - first[se]
        data[node_row[se] + slot, colbase + node_dcol[se]] = sv.astype(data.dtype)
        # evac mapping: chunk occupies evac cols [off, off+W); node value at
        # partition 32*quad + slot_idx, col off + (vcol - piece_vcol0)
        part = np.zeros(NPCL, np.int64)
        col = np.zeros(NPCL, np.int64)
        off = 0
        for W, grp in self.chunks:
            for quad, (ci, pb, v0, w) in enumerate(grp):
                cl = self.classes[ci]
                sel = np.nonzero((cls == cl) & (node_vcol >= v0) & (node_vcol < v0 + w))[0]
                part[sel] = 32 * quad + node_row[sel] // cl
                col[sel] = off + node_vcol[sel] - v0
            off += W
        return part, col


def _build_edge_program(planS, planR):
    """Launch A: deg/in_w per-node sums via mask matmuls + max(deg)."""
    nc = bacc.Bacc("TRN2", target_bir_lowering=False, debug=False,
                   num_devices=NCORES)
    A = mybir.AluOpType
    X = mybir.AxisListType.X

    ncls = len(planS.classes) + len(planR.classes)
    MASKW = 32 * ncls
    NCOLS = planS.ncols + planR.ncols
    EVC = planS.evac_cols + planR.evac_cols
    nsend = len(planS.chunks)

    ed_d = nc.dram_tensor("ed", [128, MASKW + NCOLS], fp8, kind="ExternalInput")
    degw_d = nc.dram_tensor("degw", [128, EVC], fp16, kind="ExternalOutput")
    pmax_d = nc.dram_tensor("pmax", [1, 1], f32, kind="ExternalOutput")

    # DMA split points for the edge data (4 pieces, chunk-aligned)
    allchunks = [(W, grp, True) for W, grp in planS.chunks] + \
                [(W, grp, False) for W, grp in planR.chunks]

    with tile.TileContext(nc) as tc:
        with (
            tc.tile_pool(name="ed", bufs=1) as edp,
            tc.tile_pool(name="small", bufs=1) as smallp,
            tc.tile_pool(name="ps", bufs=min(len(allchunks), 6), space="PSUM") as psp,
        ):
            ed_sb = edp.tile([128, MASKW + NCOLS], fp8)
            nc.sync.dma_start(ed_sb[:, :MASKW], ed_d[:, :MASKW])
            # split the value region into ~4 DMAs at chunk data boundaries
            bounds = [MASKW]
            # data columns are laid out class-major; chunk pieces reference
            # absolute cols.  Just split evenly into 4 and let tile dep-track.
            step = (NCOLS + 3) // 4
            for b0 in range(0, NCOLS, step):
                b1 = min(NCOLS, b0 + step)
                nc.sync.dma_start(ed_sb[:, MASKW + b0 : MASKW + b1],
                                  ed_d[:, MASKW + b0 : MASKW + b1])

            degsb = smallp.tile([128, EVC], fp16)
            dmax = smallp.tile([128, nsend], f32)
            nc.vector.memset(dmax[:, :], 0.0)

            off = 0
            sidx = 0
            for kc, (W, grp, is_send) in enumerate(allchunks):
                ps = psp.tile([128, PW], f32, tag="ps")
                nq = len(grp)
                for quad, (ci, pb, v0, w) in enumerate(grp):
                    cbase = ci if is_send else len(planS.classes) + ci
                    dcol = (0 if is_send else planS.ncols) + pb
                    nc.tensor.matmul(
                        ps[32 * quad : 32 * quad + 32, 0:W],
                        ed_sb[:, cbase * 32 : cbase * 32 + 32],
                        ed_sb[:, MASKW + dcol : MASKW + dcol + W],
                        start=True, stop=True,
                        tile_position=(0, 32 * quad),
                    )
                if is_send:
                    nc.vector.tensor_reduce(dmax[0 : 32 * nq, sidx : sidx + 1],
                                            ps[0 : 32 * nq, 0:W], axis=X, op=A.max)
                    sidx += 1
                # alternate evacuation engine so neither DVE nor ACT binds
                if kc % 2 == 0:
                    nc.scalar.activation(degsb[0 : 32 * nq, off : off + W],
                                         ps[0 : 32 * nq, 0:W],
                                         mybir.ActivationFunctionType.Identity,
                                         bias=0.0, scale=1.0)
                else:
                    nc.vector.tensor_copy(degsb[0 : 32 * nq, off : off + W],
                                          ps[0 : 32 * nq, 0:W])
                off += W

            gmax = smallp.tile([128, 1], f32)
            nc.vector.tensor_reduce(gmax[:, :], dmax[:, :], axis=X, op=A.max)
            pmax = smallp.tile([128, 1], f32)
            nc.gpsimd.partition_all_reduce(pmax[:, :], gmax[:, :], channels=128,
                                           reduce_op=bass_isa.ReduceOp.max)
            nc.sync.dma_start(degw_d[:, :], degsb[:, :])
            nc.sync.dma_start(pmax_d[:, :], pmax[0:1, 0:1])

    nc.compile()
    return nc


# --------------------------------------------------------------------------
# launch B: out^T = WA^T X^T + WB^T (sX)^T + b, fp16/fp8, transposed layout
# --------------------------------------------------------------------------

def _build_main_program(srep_fp8=True):
    nc = bacc.Bacc("TRN2", target_bir_lowering=False, debug=False,
                   num_devices=NCORES)
    A = mybir.AluOpType
    X = mybir.AxisListType.X
    sdt = fp8 if srep_fp8 else fp16

    xt_d = nc.dram_tensor("xt", [F, NPAD], fp16, kind="ExternalInput")
    wk_d = nc.dram_tensor("wk", [128, KCH * F], fp16, kind="ExternalInput")
    degw_d = nc.dram_tensor("degw", [128, 2 * T], fp16, kind="ExternalInput")
    aux_d = nc.dram_tensor("aux", [128, 8], f32, kind="ExternalInput")
    out_d = nc.dram_tensor("out", [F, NPAD], fp16, kind="ExternalOutput")

    XCH = 4                    # xt / srep / out DMA chunks
    CW = NPAD // XCH           # 3136
    GW = 448                   # matmul group width (PSUM bank = 512 f32 max)
    GPC = CW // GW             # 7 groups per chunk

    with tile.TileContext(nc) as tc:
        with (
            tc.tile_pool(name="const", bufs=1) as constp,
            tc.tile_pool(name="xt", bufs=1) as xtp,
            tc.tile_pool(name="outp", bufs=1) as outp,
            tc.tile_pool(name="small", bufs=1) as smallp,
            tc.tile_pool(name="sx", bufs=5) as sxp,
            tc.tile_pool(name="srepp", bufs=1) as srepp,
            tc.tile_pool(name="psf", bufs=7, space="PSUM") as psf,
            tc.tile_pool(name="pst", bufs=1, space="PSUM") as pst,
            tc.tile_pool(name="dram", bufs=1, space="DRAM") as dramp,
        ):
            # ---- tiny input DMAs first: they feed the s chain --------------
            with tc.high_priority():
                aux_sb = smallp.tile([128, 8], f32)
                nc.scalar.dma_start(aux_sb[:, :], aux_d[:, :])
                degw_sb = smallp.tile([128, 2 * T], fp16)
                nc.scalar.dma_start(degw_sb[:, :], degw_d[:, :])
            wk_sb = constp.tile([128, KCH * F], fp16)
            nc.sync.dma_start(wk_sb[:, :], wk_d[:, :])

            # ---- node features (sync queue, early) -------------------------
            xt_sb = []
            for c in range(XCH):
                xt_c = xtp.tile([128, CW], fp16, name=f"xt{c}")
                nc.sync.dma_start(xt_c[:, :], xt_d[:, c * CW : (c + 1) * CW])
                xt_sb.append(xt_c)

            # ---- s = (deg - inw) * recip(m), transposed to node order ------
            with tc.high_priority():
                minv = smallp.tile([128, 1], f32)
                nc.vector.reciprocal(minv[:, :], aux_sb[:, 0:1])
                s_sb = smallp.tile([128, T], f32)
                nc.vector.tensor_sub(s_sb[:, :], degw_sb[:, :T], degw_sb[:, T:])
                nc.vector.tensor_scalar_mul(s_sb[:, :], s_sb[:, :], minv[:, 0:1])
                s16 = smallp.tile([128, 128], fp16)
                nc.vector.memset(s16[:, :], 0.0)
                nc.vector.tensor_copy(s16[:, 0:T], s_sb[:, :])
                ident16 = smallp.tile([128, 128], fp16)
                masks.make_identity(nc, ident16[:, :])
            with tc.high_priority():
                ps_t = pst.tile([128, 128], fp16, tag="pst")
                nc.tensor.transpose(ps_t[:, :], s16[:, :], ident16[:, :])
                s_tr = smallp.tile([128, 128], sdt)
                nc.vector.tensor_copy(s_tr[:, :], ps_t[:, :])
                strow_d = dramp.tile([T, 128], sdt)
                nc.scalar.dma_start(strow_d[:, :], s_tr[0:T, :])

            # ---- bias column + ACT table pre-warm --------------------------
            with tc.high_priority():
                act_warm = smallp.tile([1, 1], f32)
                nc.scalar.activation(act_warm[:, :], aux_sb[0:1, 0:1],
                                     mybir.ActivationFunctionType.Identity,
                                     bias=0.0, scale=1.0)
                btot_col = smallp.tile([128, 1], f32)
                nc.vector.tensor_reduce(btot_col[:, :], aux_sb[:, 1:8],
                                        axis=X, op=A.add)

            # ---- weights: WA | WB in fp16 ----------------------------------
            # WA = W0 - W2 - 2W3 - 3W4 - 4W5,  WB = W1 + 2W2 + 3W3 + 4W4 + 5W5
            def wkk(k):
                return wk_sb[:, k * F : (k + 1) * F]
            wa16 = constp.tile([128, F], fp16)
            wb16 = constp.tile([128, F], fp16)
            nc.vector.scalar_tensor_tensor(wa16[:, :], wkk(2), -1.0, wkk(0), op0=A.mult, op1=A.add)
            nc.vector.scalar_tensor_tensor(wa16[:, :], wkk(3), -2.0, wa16[:, :], op0=A.mult, op1=A.add)
            nc.vector.scalar_tensor_tensor(wa16[:, :], wkk(4), -3.0, wa16[:, :], op0=A.mult, op1=A.add)
            nc.vector.scalar_tensor_tensor(wa16[:, :], wkk(5), -4.0, wa16[:, :], op0=A.mult, op1=A.add)
            nc.vector.scalar_tensor_tensor(wb16[:, :], wkk(2), 2.0, wkk(1), op0=A.mult, op1=A.add)
            nc.vector.scalar_tensor_tensor(wb16[:, :], wkk(3), 3.0, wb16[:, :], op0=A.mult, op1=A.add)
            nc.vector.scalar_tensor_tensor(wb16[:, :], wkk(4), 4.0, wb16[:, :], op0=A.mult, op1=A.add)
            nc.vector.scalar_tensor_tensor(wb16[:, :], wkk(5), 5.0, wb16[:, :], op0=A.mult, op1=A.add)

            # ---- s broadcast: every partition re-reads the s row -----------
            sflat = strow_d[:, :]
            srep_sb = []
            for c in range(XCH):
                srep_c = srepp.tile([128, CW], sdt, name=f"srep{c}")
                srcap = bass.AP(sflat.tensor, sflat.offset + c * CW, [[0, 128], [1, CW]])
                nc.scalar.dma_start(srep_c[:, :], srcap)
                srep_sb.append(srep_c)

            # ---- main loop -------------------------------------------------
            for c in range(XCH):
                out_c = outp.tile([128, CW], fp16, name=f"out{c}")
                srep_c = srep_sb[c]
                sxs, psFs = [], []
                for g in range(GPC):
                    n0 = g * GW
                    sx = sxp.tile([128, GW], fp16, tag="sx")
                    nc.vector.tensor_tensor(sx[:, :], xt_sb[c][:, n0 : n0 + GW],
                                            srep_c[:, n0 : n0 + GW], op=A.mult)
                    sxs.append(sx)
                for g in range(GPC):
                    n0 = g * GW
                    psF = psf.tile([128, GW], f32, tag="psf")
                    nc.tensor.matmul(psF[:, :], wa16[:, :],
                                     xt_sb[c][:, n0 : n0 + GW], start=True, stop=False)
                    psFs.append(psF)
                for g in range(GPC):
                    nc.tensor.matmul(psFs[g][:, :], wb16[:, :], sxs[g][:, :],
                                     start=False, stop=True)
                for g in range(GPC):
                    n0 = g * GW
                    nc.scalar.activation(out_c[:, n0 : n0 + GW], psFs[g][:, :],
                                         mybir.ActivationFunctionType.Identity,
                                         bias=btot_col[:, 0:1], scale=1.0)
                nc.sync.dma_start(out_d[:, c * CW : (c + 1) * CW], out_c[:, :])

    nc.compile()
    return nc


# --------------------------------------------------------------------------
# host driver
# --------------------------------------------------------------------------

def kernel(nodes, edges, senders, receivers, Wk, bk, bias):
    nodes = np.ascontiguousarray(np.asarray(nodes, np.float32))
    edges = np.ascontiguousarray(np.asarray(edges, np.float32))
    senders = np.asarray(senders)
    receivers = np.asarray(receivers)
    Wk = np.ascontiguousarray(np.asarray(Wk, np.float32))
    bk = np.asarray(bk, np.float32)
    bias = np.asarray(bias, np.float32)
    assert nodes.shape == (N_NODES, F) and Wk.shape == (KCH, F, F)

    cores = list(range(NCORES))

    # ---- common packing layout across cores (permutation + zero fill only) -
    lnS_c, lnR_c, wS_c, wR_c, clsS_c, clsR_c = [], [], [], [], [], []
    for c in cores:
        mS = (senders // NPC) == c
        mR = (receivers // NPC) == c
        lnS = (senders[mS] - c * NPC).astype(np.int64)
        lnR = (receivers[mR] - c * NPC).astype(np.int64)
        lnS_c.append(lnS); lnR_c.append(lnR)
        wS_c.append(edges[mS]); wR_c.append(edges[mR])
        for ln, dst in ((lnS, clsS_c), (lnR, clsR_c)):
            cnt = np.bincount(ln, minlength=NPC)
            dst.append(np.maximum(((np.maximum(cnt, 1) + 7) // 8) * 8, 8))
    planS = _CommonPlan(clsS_c)
    planR = _CommonPlan(clsR_c)

    ncls = len(planS.classes) + len(planR.classes)
    MASKW = 32 * ncls
    in_a = []
    evmaps = []  # (partS, colS, partR, colR)
    maskblk = np.zeros((128, MASKW), np_fp8)
    mi = 0
    for plan in (planS, planR):
        for cl in plan.classes:
            k = 128 // cl
            for i in range(k):
                maskblk[i * cl : (i + 1) * cl, mi * 32 + i] = 1.0
            mi += 1
    for c in cores:
        data = np.zeros((128, MASKW + planS.ncols + planR.ncols), np_fp8)
        data[:, :MASKW] = maskblk
        pS, cS = planS.fill(data, MASKW, lnS_c[c], wS_c[c], NPC)
        pR, cR = planR.fill(data, MASKW + planS.ncols, lnR_c[c], wR_c[c], NPC)
        evmaps.append((pS, cS, pR, cR))
        in_a.append({"ed": np.ascontiguousarray(data)})

    key = ("edge", tuple(planS.classes), tuple(planR.classes),
           planS.ncols, planR.ncols,
           tuple((W, tuple(map(tuple, g))) for W, g in planS.chunks),
           tuple((W, tuple(map(tuple, g))) for W, g in planR.chunks))
    if key not in _prog_cache:
        _prog_cache[key] = _build_edge_program(planS, planR)
    ncA = _prog_cache[key]

    res_a = run_bass_kernel_spmd(ncA, in_a, cores, trace=TRACE)

    # combine the 8 device partial maxima (selection, no arithmetic)
    m = max(float(res_a.results[c]["pmax"][0, 0]) for c in cores)

    # ---- host permutes packed sums into node order (selection only) -------
    if ("main",) not in _prog_cache:
        _prog_cache[("main",)] = _build_main_program()
    ncB = _prog_cache[("main",)]

    bkvec = np.concatenate([bk, bias.reshape(1, F)], axis=0)  # [7, F]
    wk16 = np.ascontiguousarray(
        Wk.transpose(1, 0, 2).reshape(128, KCH * F).astype(np.float16))
    in_b = []
    for c in cores:
        pS, cS, pR, cR = evmaps[c]
        dw = res_a.results[c]["degw"]                 # [128, EVC] fp16
        degw = np.zeros((128, 2 * T), np.float16)
        node = np.arange(NPC)
        degw[node % 128, node // 128] = dw[pS, cS]
        degw[node % 128, T + node // 128] = dw[pR, cR + planS.evac_cols]
        aux = np.zeros((128, 8), np.float32)
        aux[:, 0] = m
        aux[:, 1:8] = bkvec.T                          # [128(fo), 7]
        xt = np.zeros((F, NPAD), np.float16)
        xt[:, :NPC] = nodes[c * NPC : (c + 1) * NPC].T
        in_b.append({"xt": xt, "wk": wk16, "degw": degw, "aux": aux})
    res_b = run_bass_kernel_spmd(ncB, in_b, cores, trace=TRACE)

    ta = res_a.exec_time_ns
    tb = res_b.exec_time_ns
    LAST["exec_a_ns"] = ta
    LAST["exec_b_ns"] = tb
    LAST["exec_time_ns"] = (ta + tb) if (ta is not None and tb is not None) else None

    out = np.empty((N_NODES, F), np.float32)
    for c in cores:
        o = res_b.results[c]["out"]
        out[c * NPC : (c + 1) * NPC] = o.astype(np.float32).T[:NPC]
    return out
